# revision 2
# baseline (speedup 1.0000x reference)
"""Fixed-point (MPC) 3x3 VALID conv2d, NHWC, f32 — Trainium2 Bass kernel. v7.

Semantics (bit-exact vs the jax reference, fixed_point=8, S=256):
    qx = round_half_even(x*S)/S ; qw = round_half_even(w*S)/S
    y  = conv2d_valid(qx, qw)   ; out = floor(y*S)/S

v7 strategy per core (data-parallel over batch, 4 images/core):
  - HOST pre-quantizes x to fp16 integers (round(x*256), |.|<2048 so
    exact in fp16) and pre-builds the blocked-transposed layout
    xq[(8dw,16c)=128p, (37blk, 224h)] per image.  Device reads 8.49MB
    fp16 instead of 12.85MB f32 and does NO quantize / NO PE transpose.
  - FLIPPED banded matmul (as v6): lhsT = xq block slice [128, hc]
    (stationary, Fast Weight Load), rhs = wb[kh] [128, 96] fp16
    (moving); 3 kh taps accumulate in PSUM -> psy[h' part, (6w',16k)]
    — output lands directly in store orientation.
  - floor -> int16 (2-op): ACT u = psy/256 - 255/512 (exact f32);
    DVE st = ((u + 12582912) - 12582912) cast int16 (f32 RNE at ulp 1
    between ops = floor; result is an exact integer so the final int16
    cast is exact under any rounding mode).  int16 store halves output
    traffic: 6.31MB instead of 12.62MB.
  - HOST converts y_int16 -> f32 * (1/256) (exact) and gathers.

Per-core HBM traffic 14.8MB (vs 25.5MB in v6); PE pure matmul
85248 cyc ~= 35.5us @2.4GHz.
"""

import numpy as np

import concourse.mybir as mybir
from concourse import bass, tile

N_CORES = 8
B_FULL = 32
B_CORE = B_FULL // N_CORES  # 4 images per core
H = W = 224
C = K = 16
HO = WO = 222

F32 = mybir.dt.float32
F16 = mybir.dt.float16
I16 = mybir.dt.int16

C_RND = 12582912.0  # 1.5 * 2**23: magic addend, RNE-to-integer for |v| < 2**22
INV_S = 1.0 / 256.0
U_BIAS = -255.0 / 512.0  # makes RNE-to-integer == floor(psy/256), no ties

N_BLK = 37   # 37 blocks x 6 output w's = 222
GRP = 5      # blocks per PSUM group: 37 = 7*5 + 2
XCOL = N_BLK * H  # 8288 columns per image in the blocked-transposed layout

# block-aligned input DMA splits (cols of 224 each); groups of 5 blocks
# never span a split boundary (splits at block 10, 20, 30)
SPLITS = ((0, 10), (10, 10), (20, 10), (30, 7))

hchunks = ((0, 128), (128, 94))  # output h' chunks covering 222
groups = [(GRP * g, min(GRP, N_BLK - GRP * g))
          for g in range((N_BLK + GRP - 1) // GRP)]


def _split_multi_waits(nc):
    """The installed walrus only encodes ONE sync wait per instruction.
    Hoist extra waits onto NoOps inserted just before, same engine."""
    for f in nc.m.functions:
        for bb in f.blocks:
            new_list = []
            changed = False
            for ins in bb.instructions:
                si = ins.sync_info
                if si is not None and si.on_wait and len(si.on_wait) > 1:
                    waits = list(si.on_wait)
                    for wt in waits[:-1]:
                        nop = mybir.InstNoOp(
                            name=f"NOPW-{nc.next_id()}", ins=[], outs=[]
                        )
                        nop.engine = ins.engine
                        nop.sync_info = mybir.SyncInfo(on_wait=[wt], on_update=[])
                        new_list.append(nop)
                    ins.sync_info = mybir.SyncInfo(
                        on_wait=[waits[-1]], on_update=list(si.on_update or [])
                    )
                    changed = True
                new_list.append(ins)
            if changed:
                bb.instructions = new_list


def _build_nc(stage_limit: int = 7, reps: int = 1):
    # stage_limit: 1=loads 4=+conv 5=+floor-u 6=+floor-st 7=+store (full).
    # reps>1 repeats the whole pipeline in-NEFF (timing harness only).
    nc = bass.Bass("TRN2", num_devices=N_CORES)
    xq_d = nc.dram_tensor("xq", [B_CORE * 128, XCOL], F16, kind="ExternalInput")
    wb_d = nc.dram_tensor("wb", [3, 128, 96], F16, kind="ExternalInput")
    y_d = nc.dram_tensor("y", [B_CORE, HO, WO * K], I16, kind="ExternalOutput")

    add = mybir.AluOpType.add
    COPY = mybir.ActivationFunctionType.Copy

    with tile.TileContext(nc) as tc:
        with (
            tc.tile_pool(name="consts", bufs=1) as consts,
            tc.tile_pool(name="xq", bufs=2) as xq_pool,
            tc.tile_pool(name="v", bufs=3) as v_pool,
            tc.tile_pool(name="st", bufs=2) as st_pool,
            tc.tile_pool(name="psy", bufs=5, space="PSUM") as ps_y_pool,
        ):
            wtiles = []
            for kh in range(3):
                wt = consts.tile([128, 96], F16, tag=f"w{kh}")
                nc.sync.dma_start(out=wt[:], in_=wb_d[kh])
                wtiles.append(wt)

            for rp in range(reps):
                for img in range(B_CORE):
                    # ---- input DMA: 4 block-aligned stripes ----
                    xt = {}  # block -> (tile, col base within tile)
                    for si, (b0, nb) in enumerate(SPLITS):
                        t = xq_pool.tile([128, nb * H], F16, tag=f"xq{si}")
                        nc.sync.dma_start(
                            out=t[:],
                            in_=xq_d[128 * img : 128 * img + 128,
                                     H * b0 : H * (b0 + nb)],
                        )
                        for b in range(b0, b0 + nb):
                            xt[b] = (t, H * (b - b0))
                    if stage_limit < 4:
                        continue
                    for ch, (h0, hc) in enumerate(hchunks):
                        st_t = st_pool.tile([128, N_BLK * 96], I16,
                                            tag=f"st{ch}")
                        for g0, gn in groups:
                            psy = ps_y_pool.tile([128, GRP, 96], F32,
                                                 tag="psy")
                            for b in range(g0, g0 + gn):
                                t, cb = xt[b]
                                for s in range(3):
                                    nc.tensor.matmul(
                                        out=psy[:hc, b - g0, :],
                                        lhsT=t[:, cb + h0 + s :
                                               cb + h0 + s + hc],
                                        rhs=wtiles[s][:],
                                        start=(s == 0),
                                        stop=(s == 2),
                                    )
                            if stage_limit >= 5:
                                cols = slice(96 * g0, 96 * (g0 + gn))
                                v1 = v_pool.tile([128, GRP, 96], F32,
                                                 tag="v1")
                                nc.scalar.activation(
                                    out=v1[:hc, :gn, :],
                                    in_=psy[:hc, :gn, :],
                                    func=COPY, bias=U_BIAS, scale=INV_S,
                                )
                                if stage_limit >= 6:
                                    nc.vector.tensor_scalar(
                                        out=st_t[:hc, cols],
                                        in0=v1[:hc, :gn, :],
                                        scalar1=C_RND, scalar2=-C_RND,
                                        op0=add, op1=add,
                                    )
                            if stage_limit >= 7 and g0 + gn == 20:
                                # first 4 groups' columns are final: stream
                                # the front of this chunk's store now
                                nc.gpsimd.dma_start(
                                    out=y_d[img, h0 : h0 + hc, : 20 * 96],
                                    in_=st_t[:hc, : 20 * 96],
                                )
                        if stage_limit >= 7:
                            nc.gpsimd.dma_start(
                                out=y_d[img, h0 : h0 + hc, 20 * 96 :],
                                in_=st_t[:hc, 20 * 96 :],
                            )

    _split_multi_waits(nc)
    return nc


def _banded_weights(w: np.ndarray) -> np.ndarray:
    """w [3,3,16,16] f32 -> wb [3, 128, 96] fp16 banded lhsT matrices.

    wb[kh][16*dw + c, 16*j + k] = round(w*256)[kh, dw - j, c, k]
    for 0 <= dw - j <= 2, j in 0..5."""
    wq = np.round(w.astype(np.float32) * np.float32(256.0))  # RNE, exact
    assert np.abs(wq).max() < 2048, "w_int exceeds fp16-exact budget"
    wb = np.zeros((3, 128, 96), dtype=np.float32)
    for kh in range(3):
        for j in range(6):
            for kw in range(3):
                dw = j + kw
                wb[kh, 16 * dw : 16 * dw + 16, 16 * j : 16 * j + 16] = wq[kh, kw]
    return wb.astype(np.float16)


def _blocked_x(x: np.ndarray) -> np.ndarray:
    """x [32,224,224,16] f32 -> xq [32, 128, 37*224] fp16.

    xq[img, 16*dw + c, 224*b + h] = round(x*256)[img, h, 6*b + dw, c]."""
    qx = np.round(x * np.float32(256.0)).astype(np.float16)  # RNE, exact
    sw = np.lib.stride_tricks.sliding_window_view(qx, 8, axis=2)
    # sw [img, h, wstart, c, dw] -> take starts 0,6,...,216
    sw = sw[:, :, ::6, :, :]                      # [32, 224, 37, 16, 8]
    xq = sw.transpose(0, 4, 3, 2, 1)              # [32, 8, 16, 37, 224]
    return np.ascontiguousarray(xq).reshape(B_FULL, 128, XCOL)


_RUNNER = None


def _get_runner():
    global _RUNNER
    if _RUNNER is None:
        _RUNNER = _make_runner(_build_nc())
    return _RUNNER


def _make_runner(nc):
    """Mirrors concourse.bass2jax.run_bass_via_pjrt's multi-core path but
    caches the jitted executable so repeated calls don't recompile."""
    import jax
    from jax.sharding import Mesh, PartitionSpec
    from jax.experimental.shard_map import shard_map
    from concourse.bass2jax import (
        _bass_exec_p,
        install_neuronx_cc_hook,
        partition_id_tensor,
    )

    install_neuronx_cc_hook()

    partition_name = nc.partition_id_tensor.name if nc.partition_id_tensor else None
    in_names, out_names, out_avals, zero_outs = [], [], [], []
    for alloc in nc.m.functions[0].allocations:
        if not isinstance(alloc, mybir.MemoryLocationSet):
            continue
        name = alloc.memorylocations[0].name
        if alloc.kind == "ExternalInput":
            if name != partition_name:
                in_names.append(name)
        elif alloc.kind == "ExternalOutput":
            out_names.append(name)
            shape = tuple(alloc.tensor_shape)
            dtype = mybir.dt.np(alloc.dtype)
            out_avals.append(jax.core.ShapedArray(shape, dtype))
            zero_outs.append(np.zeros(shape, dtype))
    n_params = len(in_names)
    n_outs = len(out_avals)
    all_in_names = list(in_names) + list(out_names)
    if partition_name is not None:
        all_in_names.append(partition_name)

    def _body(*args):
        operands = list(args)
        if partition_name is not None:
            operands.append(partition_id_tensor())
        outs = _bass_exec_p.bind(
            *operands,
            out_avals=tuple(out_avals),
            in_names=tuple(all_in_names),
            out_names=tuple(out_names),
            lowering_input_output_aliases=(),
            sim_require_finite=True,
            sim_require_nnan=True,
            nc=nc,
        )
        return tuple(outs)

    devices = jax.devices()[:N_CORES]
    assert len(devices) == N_CORES, f"need {N_CORES} devices, got {len(devices)}"
    mesh = Mesh(np.asarray(devices), ("core",))
    in_specs = (PartitionSpec("core"),) * (n_params + n_outs)
    out_specs = (PartitionSpec("core"),) * n_outs
    sharded = jax.jit(
        shard_map(_body, mesh=mesh, in_specs=in_specs, out_specs=out_specs,
                  check_rep=False),
        donate_argnums=tuple(range(n_params, n_params + n_outs)),
        keep_unused=True,
    )

    state = {
        "sharded": sharded,
        "in_names": in_names,
        "out_names": out_names,
        "out_avals": out_avals,
        "zero_outs": zero_outs,
        "n_cores": N_CORES,
    }

    def runner(in_maps):
        per_core = [[np.asarray(m[nm]) for nm in in_names] for m in in_maps]
        concat_in = [
            np.concatenate([per_core[c][i] for c in range(N_CORES)], axis=0)
            for i in range(n_params)
        ]
        concat_zeros = [
            np.concatenate([z] * N_CORES, axis=0) for z in zero_outs
        ]
        out_arrs = state["sharded"](*concat_in, *concat_zeros)
        return [
            {
                nm: np.asarray(out_arrs[i]).reshape(
                    N_CORES, *out_avals[i].shape
                )[c]
                for i, nm in enumerate(out_names)
            }
            for c in range(N_CORES)
        ]

    runner.state = state
    return runner


def _make_in_maps(x: np.ndarray, w: np.ndarray) -> list[dict]:
    wb = _banded_weights(np.asarray(w, dtype=np.float32))
    xq = _blocked_x(np.asarray(x, dtype=np.float32))
    in_maps = []
    for core in range(N_CORES):
        xs = xq[B_CORE * core : B_CORE * (core + 1)].reshape(B_CORE * 128, XCOL)
        in_maps.append({"xq": xs, "wb": wb})
    return in_maps


def _assemble_out(results: list[dict]) -> np.ndarray:
    y16 = np.concatenate([r["y"] for r in results], axis=0)  # [32, 222, 3552]
    y = y16.astype(np.float32) * np.float32(1.0 / 256.0)
    return np.ascontiguousarray(y.reshape(B_FULL, HO, WO, K))


def kernel(x: np.ndarray, w: np.ndarray, fixed_point) -> np.ndarray:
    assert int(fixed_point) == 8, f"kernel hardcodes fixed_point=8, got {fixed_point}"
    x = np.ascontiguousarray(np.asarray(x, dtype=np.float32))
    assert x.shape == (B_FULL, H, W, C), x.shape
    assert np.abs(x).max() * 256.0 < 2040.0, "x_int exceeds fp16-exact budget"

    runner = _get_runner()
    results = runner(_make_in_maps(x, w))
    return _assemble_out(results)


# revision 4
# speedup vs baseline: 263.7622x; 263.7622x over previous
"""Fixed-point (MPC) 3x3 VALID conv2d, NHWC, f32 — Trainium2 Bass kernel. v8.

Semantics (bit-exact vs the jax reference, fixed_point=8, S=256):
    qx = round_half_even(x*S)/S ; qw = round_half_even(w*S)/S
    y  = conv2d_valid(qx, qw)   ; out = floor(y*S)/S

v8 strategy per core (data-parallel over batch, 4 images/core):
  - HOST pre-quantizes x to fp16 integers (round(x*256), |.|<2048 so
    exact in fp16) and builds an image-PACKED blocked-transposed
    layout: xq[(8dw,16c)=128p, (37blk, 4img*224h + 2pad)] — the 4
    images are contiguous within each block stripe, so the 888 output
    rows per block are covered by SEVEN M=128 stationary windows
    (vs 8 for per-image chunking), all with NumWeights==128 => Fast
    Weight Load stays enabled on every matmul.  Seam partitions
    (output h' in {222,223}) are garbage and never stored.
  - FLIPPED banded matmul: lhsT = xq window [128, 128] (stationary,
    FWL), rhs = wb[kh] [128, 96] fp16 (moving, PRE-SCALED by 1/256 so
    PSUM holds y on the 2^-8 grid exactly); 3 kh taps accumulate in
    PSUM -> psy[row part, (6w',16k)] in store orientation.
  - floor -> int16 in ONE DVE op: int16_cast_RNE(psy - 255/512)
    == floor(psy*256) exactly (no ties; HW cast verified RNE).
    Scalar engine is entirely free; int16 store halves output traffic.
  - HOST converts y_int16 -> f32 * (1/256) (exact) and gathers.

Per-core HBM traffic 14.8MB; PE 777 matmuls ~= 41us; DVE ~28us.
"""

import numpy as np

import concourse.mybir as mybir
from concourse import bass, tile

N_CORES = 8
B_FULL = 32
B_CORE = B_FULL // N_CORES  # 4 images per core
H = W = 224
C = K = 16
HO = WO = 222

F32 = mybir.dt.float32
F16 = mybir.dt.float16
I16 = mybir.dt.int16

FLOOR_C = -255.0 / 512.0  # RNE(v + FLOOR_C) == floor(v) for v on 2^-8 grid

N_BLK = 37       # 37 blocks x 6 output w's = 222
GRP = 5          # blocks per PSUM group: 37 = 7*5 + 2
PACK = B_CORE * H          # 896 packed rows per block stripe
XSTRIDE = PACK + 2         # 898: 2 zero pad cols (window 6 taps 1,2)
XCOL = N_BLK * XSTRIDE     # 33226 columns total
N_WIN = 7                  # ceil(896 / 128) stationary windows

# block groups of 5 (last 2); input DMA split == groups
groups = [(GRP * g, min(GRP, N_BLK - GRP * g))
          for g in range((N_BLK + GRP - 1) // GRP)]

# window w -> contiguous valid-output runs (i0, n, img, h0):
# partition i of window w is global packed row g = 128*w + i,
# img = g // 224, h' = g % 224, valid while h' <= 221.
WMAP = []
for _w in range(N_WIN):
    runs = []
    _i = 0
    while _i < 128:
        _g = 128 * _w + _i
        _img, _hp = divmod(_g, 224)
        if _hp >= HO:
            _i += 1
            continue
        _n = min(128 - _i, HO - _hp)
        runs.append((_i, _n, _img, _hp))
        _i += _n
    WMAP.append(runs)


def _split_multi_waits(nc):
    """The installed walrus only encodes ONE sync wait per instruction.
    Hoist extra waits onto NoOps inserted just before, same engine."""
    for f in nc.m.functions:
        for bb in f.blocks:
            new_list = []
            changed = False
            for ins in bb.instructions:
                si = ins.sync_info
                if si is not None and si.on_wait and len(si.on_wait) > 1:
                    waits = list(si.on_wait)
                    for wt in waits[:-1]:
                        nop = mybir.InstNoOp(
                            name=f"NOPW-{nc.next_id()}", ins=[], outs=[]
                        )
                        nop.engine = ins.engine
                        nop.sync_info = mybir.SyncInfo(on_wait=[wt], on_update=[])
                        new_list.append(nop)
                    ins.sync_info = mybir.SyncInfo(
                        on_wait=[waits[-1]], on_update=list(si.on_update or [])
                    )
                    changed = True
                new_list.append(ins)
            if changed:
                bb.instructions = new_list


def _build_nc(stage_limit: int = 7, reps: int = 1):
    # stage_limit: 1=loads 4=+conv 6=+floor 7=+store (full kernel).
    # reps>1 repeats the whole pipeline in-NEFF (timing harness only).
    nc = bass.Bass("TRN2", num_devices=N_CORES)
    xq_d = nc.dram_tensor("xq", [128, XCOL], F16, kind="ExternalInput")
    wb_d = nc.dram_tensor("wb", [3, 128, 96], F16, kind="ExternalInput")
    y_d = nc.dram_tensor("y", [B_CORE, HO, WO * K], I16, kind="ExternalOutput")

    add = mybir.AluOpType.add

    with tile.TileContext(nc) as tc:
        with (
            tc.tile_pool(name="consts", bufs=1) as consts,
            tc.tile_pool(name="xq", bufs=1) as xq_pool,
            tc.tile_pool(name="st", bufs=2) as st_pool,
            tc.tile_pool(name="psy", bufs=5, space="PSUM") as ps_pool,
        ):
            wtiles = []
            for kh in range(3):
                wt = consts.tile([128, 96], F16, tag=f"w{kh}")
                nc.sync.dma_start(out=wt[:], in_=wb_d[kh])
                wtiles.append(wt)

            for rp in range(reps):
                # ---- input DMA: one stripe-aligned split per group ----
                xts = []  # per group: (tile, block base)
                for gi, (b0, nb) in enumerate(groups):
                    t = xq_pool.tile([128, nb * XSTRIDE], F16, tag=f"xq{gi}")
                    nc.sync.dma_start(
                        out=t[:],
                        in_=xq_d[:, XSTRIDE * b0 : XSTRIDE * (b0 + nb)],
                    )
                    xts.append((t, b0))
                if stage_limit < 4:
                    continue

                st_tiles = []
                for w in range(N_WIN):
                    st_w = st_pool.tile([128, N_BLK * 96], I16, tag=f"st{w}",
                                        name=f"st{w}")
                    st_tiles.append(st_w)

                for gi, (b0, gn) in enumerate(groups):
                    t, _ = xts[gi]
                    for w in range(N_WIN):
                        psy = ps_pool.tile([128, GRP, 96], F32, tag="psy")
                        for b in range(gn):
                            cb = XSTRIDE * b + 128 * w
                            for s in range(3):
                                nc.tensor.matmul(
                                    out=psy[:, b, :],
                                    lhsT=t[:, cb + s : cb + s + 128],
                                    rhs=wtiles[s][:],
                                    start=(s == 0),
                                    stop=(s == 2),
                                )
                        if stage_limit >= 6:
                            nc.vector.tensor_scalar(
                                out=st_tiles[w][:, 96 * b0 : 96 * (b0 + gn)],
                                in0=psy[:, :gn, :],
                                scalar1=FLOOR_C, scalar2=None, op0=add,
                            )
                    if stage_limit >= 7 and b0 + gn == 20:
                        # first 4 groups' columns are final: stream the
                        # front half of every window's store now
                        for w in range(N_WIN):
                            for (i0, n, img, h0) in WMAP[w]:
                                nc.gpsimd.dma_start(
                                    out=y_d[img, h0 : h0 + n, : 20 * 96],
                                    in_=st_tiles[w][i0 : i0 + n, : 20 * 96],
                                )
                if stage_limit >= 7:
                    for w in range(N_WIN):
                        for (i0, n, img, h0) in WMAP[w]:
                            nc.gpsimd.dma_start(
                                out=y_d[img, h0 : h0 + n, 20 * 96 :],
                                in_=st_tiles[w][i0 : i0 + n, 20 * 96 :],
                            )

    _split_multi_waits(nc)
    return nc


def _banded_weights(w: np.ndarray) -> np.ndarray:
    """w [3,3,16,16] f32 -> wb [3, 128, 96] fp16 banded lhsT matrices,
    PRE-SCALED by 1/256 (exact in fp16: just an exponent shift).

    wb[kh][16*dw + c, 16*j + k] = round(w*256)[kh, dw - j, c, k] / 256
    for 0 <= dw - j <= 2, j in 0..5."""
    wq = np.round(w.astype(np.float32) * np.float32(256.0))  # RNE, exact
    assert np.abs(wq).max() < 2048, "w_int exceeds fp16-exact budget"
    wb = np.zeros((3, 128, 96), dtype=np.float32)
    for kh in range(3):
        for j in range(6):
            for kw in range(3):
                dw = j + kw
                wb[kh, 16 * dw : 16 * dw + 16, 16 * j : 16 * j + 16] = wq[kh, kw]
    return (wb * np.float32(1.0 / 256.0)).astype(np.float16)


def _blocked_x(x: np.ndarray) -> np.ndarray:
    """x [32,224,224,16] f32 -> xq [8 cores, 128, XCOL] fp16, where
    xq[core, 16*dw+c, XSTRIDE*b + 224*img + h]
        = round(x*256)[4*core+img, h, 6*b + dw, c]  (pad cols zero)."""
    qx = np.round(x * np.float32(256.0)).astype(np.float16)  # RNE, exact
    sw = np.lib.stride_tricks.sliding_window_view(qx, 8, axis=2)
    sw = sw[:, :, ::6, :, :]                     # [32, 224h, 37b, 16c, 8dw]
    sw = sw.reshape(N_CORES, B_CORE, H, N_BLK, C, 8)
    xq = sw.transpose(0, 5, 4, 3, 1, 2)          # [8, 8dw, 16c, 37b, 4, 224]
    xq = np.ascontiguousarray(xq).reshape(N_CORES, 128, N_BLK, PACK)
    xq = np.pad(xq, ((0, 0), (0, 0), (0, 0), (0, XSTRIDE - PACK)))
    return xq.reshape(N_CORES, 128, XCOL)


_RUNNER = None


def _get_runner():
    global _RUNNER
    if _RUNNER is None:
        _RUNNER = _make_runner(_build_nc())
    return _RUNNER


def _make_runner(nc):
    """Mirrors concourse.bass2jax.run_bass_via_pjrt's multi-core path but
    caches the jitted executable so repeated calls don't recompile."""
    import jax
    from jax.sharding import Mesh, PartitionSpec
    from jax.experimental.shard_map import shard_map
    from concourse.bass2jax import (
        _bass_exec_p,
        install_neuronx_cc_hook,
        partition_id_tensor,
    )

    install_neuronx_cc_hook()

    partition_name = nc.partition_id_tensor.name if nc.partition_id_tensor else None
    in_names, out_names, out_avals, zero_outs = [], [], [], []
    for alloc in nc.m.functions[0].allocations:
        if not isinstance(alloc, mybir.MemoryLocationSet):
            continue
        name = alloc.memorylocations[0].name
        if alloc.kind == "ExternalInput":
            if name != partition_name:
                in_names.append(name)
        elif alloc.kind == "ExternalOutput":
            out_names.append(name)
            shape = tuple(alloc.tensor_shape)
            dtype = mybir.dt.np(alloc.dtype)
            out_avals.append(jax.core.ShapedArray(shape, dtype))
            zero_outs.append(np.zeros(shape, dtype))
    n_params = len(in_names)
    n_outs = len(out_avals)
    all_in_names = list(in_names) + list(out_names)
    if partition_name is not None:
        all_in_names.append(partition_name)

    def _body(*args):
        operands = list(args)
        if partition_name is not None:
            operands.append(partition_id_tensor())
        outs = _bass_exec_p.bind(
            *operands,
            out_avals=tuple(out_avals),
            in_names=tuple(all_in_names),
            out_names=tuple(out_names),
            lowering_input_output_aliases=(),
            sim_require_finite=True,
            sim_require_nnan=True,
            nc=nc,
        )
        return tuple(outs)

    devices = jax.devices()[:N_CORES]
    assert len(devices) == N_CORES, f"need {N_CORES} devices, got {len(devices)}"
    mesh = Mesh(np.asarray(devices), ("core",))
    in_specs = (PartitionSpec("core"),) * (n_params + n_outs)
    out_specs = (PartitionSpec("core"),) * n_outs
    sharded = jax.jit(
        shard_map(_body, mesh=mesh, in_specs=in_specs, out_specs=out_specs,
                  check_rep=False),
        donate_argnums=tuple(range(n_params, n_params + n_outs)),
        keep_unused=True,
    )

    state = {
        "sharded": sharded,
        "in_names": in_names,
        "out_names": out_names,
        "out_avals": out_avals,
        "zero_outs": zero_outs,
        "n_cores": N_CORES,
    }

    def runner(in_maps):
        per_core = [[np.asarray(m[nm]) for nm in in_names] for m in in_maps]
        concat_in = [
            np.concatenate([per_core[c][i] for c in range(N_CORES)], axis=0)
            for i in range(n_params)
        ]
        concat_zeros = [
            np.concatenate([z] * N_CORES, axis=0) for z in zero_outs
        ]
        out_arrs = state["sharded"](*concat_in, *concat_zeros)
        return [
            {
                nm: np.asarray(out_arrs[i]).reshape(
                    N_CORES, *out_avals[i].shape
                )[c]
                for i, nm in enumerate(out_names)
            }
            for c in range(N_CORES)
        ]

    runner.state = state
    return runner


def _make_in_maps(x: np.ndarray, w: np.ndarray) -> list[dict]:
    wb = _banded_weights(np.asarray(w, dtype=np.float32))
    xq = _blocked_x(np.asarray(x, dtype=np.float32))
    return [{"xq": xq[core], "wb": wb} for core in range(N_CORES)]


def _assemble_out(results: list[dict]) -> np.ndarray:
    y16 = np.concatenate([r["y"] for r in results], axis=0)  # [32, 222, 3552]
    y = y16.astype(np.float32) * np.float32(1.0 / 256.0)
    return np.ascontiguousarray(y.reshape(B_FULL, HO, WO, K))


def kernel(x: np.ndarray, w: np.ndarray, fixed_point) -> np.ndarray:
    assert int(fixed_point) == 8, f"kernel hardcodes fixed_point=8, got {fixed_point}"
    x = np.ascontiguousarray(np.asarray(x, dtype=np.float32))
    assert x.shape == (B_FULL, H, W, C), x.shape
    assert np.abs(x).max() * 256.0 < 2040.0, "x_int exceeds fp16-exact budget"

    runner = _get_runner()
    results = runner(_make_in_maps(x, w))
    return _assemble_out(results)
